# revision 1
# baseline (speedup 1.0000x reference)
"""Trainium2 Bass kernel for nn_Classify_MLPPredictor (edge-parallel GNN inference).

Computes sigmoid(cat([h[src], h[dst]], -1) @ W + b) for E=1.6M edges over a
N=100k x 128 node table, on 8 NeuronCores.

Algorithm (per core, edges sharded 200k/core, h/W/b replicated):
  Phase 1: Pcat = h @ [Ws | Wd] + [0 | b]  -> two DRAM tables ps, pd [100k, 128]
           (factored form: avoids per-edge matmuls; each node row is reused
           ~16x by the gather phase).
  Phase 2: per 128-edge tile, indirect-DMA gather ps[src], pd[dst] into SBUF,
           add, sigmoid, write out rows.
"""

import os
import time

import numpy as np

import concourse.bass as bass
import concourse.bacc as bacc
import concourse.mybir as mybir
import concourse.tile as tile
from concourse.bass_utils import run_bass_kernel_spmd

N_CORES = 8
N_NODES = 100000
D = 128           # feature dim
C = 128           # classes
CC = 2 * C        # concatenated output cols of phase 1
E = 1600000
E_C = E // N_CORES            # 200000 edges per core

# phase 1 tiling
P1_CHUNK = 1024               # nodes per DMA chunk (8 matmul subtiles)

# phase 2 tiling
TILE_E = 128                  # edges per gather
TILES_PER_BLK = 32            # gathers fused into one add/sigmoid/store block
BLK_E = TILE_E * TILES_PER_BLK  # 4096

N_TILES = (E_C + TILE_E - 1) // TILE_E          # 1563 (last has 64 edges)
IDX_COLS = N_TILES                               # idx sbuf layout [128, N_TILES]

F32 = mybir.dt.float32
I32 = mybir.dt.int32

_CACHE = {}


def _build_program(repeat=1):
    nc = bacc.Bacc(None, target_bir_lowering=False)

    ht = nc.dram_tensor("ht", [D, N_NODES], F32, kind="ExternalInput")
    wcat = nc.dram_tensor("wcat", [D, CC], F32, kind="ExternalInput")
    bcat = nc.dram_tensor("bcat", [128, CC], F32, kind="ExternalInput")
    src_idx = nc.dram_tensor("src_idx", [128, IDX_COLS], I32, kind="ExternalInput")
    dst_idx = nc.dram_tensor("dst_idx", [128, IDX_COLS], I32, kind="ExternalInput")
    out = nc.dram_tensor("out", [E_C, C], F32, kind="ExternalOutput")

    ps = nc.dram_tensor("ps", [N_NODES, C], F32, kind="Internal")
    pd = nc.dram_tensor("pd", [N_NODES, C], F32, kind="Internal")

    with tile.TileContext(nc) as tc:
        with (
            tc.tile_pool(name="const", bufs=1) as cpool,
            tc.tile_pool(name="p1x", bufs=2) as xpool,
            tc.tile_pool(name="p1s", bufs=2) as spool,
            tc.tile_pool(name="psum", bufs=4, space="PSUM") as psum,
            tc.tile_pool(name="idx", bufs=1) as ipool,
            tc.tile_pool(name="g", bufs=2) as gpool,
            tc.tile_pool(name="o", bufs=2) as opool,
        ):
            wcat_t = cpool.tile([D, CC], F32)
            nc.sync.dma_start(out=wcat_t[:], in_=wcat[:])
            bcat_t = cpool.tile([128, CC], F32)
            nc.sync.dma_start(out=bcat_t[:], in_=bcat[:])

            # load all phase-2 indices up front (overlaps with phase 1)
            src_sb = ipool.tile([128, IDX_COLS], I32, tag="sidx")
            dst_sb = ipool.tile([128, IDX_COLS], I32, tag="didx")
            nc.sync.dma_start(out=src_sb[:], in_=src_idx[:])
            nc.sync.dma_start(out=dst_sb[:], in_=dst_idx[:])

            import contextlib

            rep_ctx = (
                tc.For_i(0, repeat, 1) if repeat > 1 else contextlib.nullcontext()
            )
            with rep_ctx:
                _emit_body(
                    nc, tc, xpool, spool, psum, gpool, opool,
                    ht, wcat_t, bcat_t, src_sb, dst_sb, ps, pd, out,
                )

    nc.compile()
    return nc


def _emit_body(nc, tc, xpool, spool, psum, gpool, opool,
               ht, wcat_t, bcat_t, src_sb, dst_sb, ps, pd, out):
    if True:
        if True:

            # ---------------- Phase 1: ps/pd = h @ [Ws|Wd] + [0|b] ----------------
            n0 = 0
            while n0 < N_NODES:
                nn = min(P1_CHUNK, N_NODES - n0)
                nsub = (nn + 127) // 128
                x = xpool.tile([D, P1_CHUNK], F32, tag="x")
                nc.sync.dma_start(out=x[:, :nn], in_=ht[:, n0 : n0 + nn])
                s = spool.tile([128, (P1_CHUNK // 128) * CC], F32, tag="s")
                for si in range(nsub):
                    m = min(128, nn - si * 128)
                    acc = psum.tile([128, CC], F32, tag="acc", space="PSUM")
                    nc.tensor.matmul(
                        acc[:m, :],
                        lhsT=x[:, si * 128 : si * 128 + m],
                        rhs=wcat_t[:],
                        start=True,
                        stop=True,
                    )
                    nc.vector.tensor_add(
                        out=s[:m, si * CC : (si + 1) * CC],
                        in0=acc[:m, :],
                        in1=bcat_t[:m, :],
                    )
                if nn == P1_CHUNK:
                    sv = s[:].rearrange("p (s q) -> p s q", s=nsub)
                    nc.sync.dma_start(
                        out=ps[n0 : n0 + nn, :].rearrange("(s p) c -> p s c", p=128),
                        in_=sv[:, :, 0:C],
                    )
                    nc.sync.dma_start(
                        out=pd[n0 : n0 + nn, :].rearrange("(s p) c -> p s c", p=128),
                        in_=sv[:, :, C:CC],
                    )
                else:
                    for si in range(nsub):
                        m = min(128, nn - si * 128)
                        r0 = n0 + si * 128
                        nc.sync.dma_start(
                            out=ps[r0 : r0 + m, :],
                            in_=s[:m, si * CC : si * CC + C],
                        )
                        nc.sync.dma_start(
                            out=pd[r0 : r0 + m, :],
                            in_=s[:m, si * CC + C : (si + 1) * CC],
                        )
                n0 += nn

            # ---------------- Phase 2: gather + add + sigmoid + store -------------
            t = 0
            while t < N_TILES:
                nt = min(TILES_PER_BLK, N_TILES - t)
                blk_w = nt * TILE_E
                gs = gpool.tile([128, BLK_E], F32, tag="gs")
                gd = gpool.tile([128, BLK_E], F32, tag="gd")
                for i in range(nt):
                    tt = t + i
                    pp = min(TILE_E, E_C - tt * TILE_E)
                    nc.gpsimd.indirect_dma_start(
                        out=gs[:pp, i * C : (i + 1) * C],
                        out_offset=None,
                        in_=ps[:, :],
                        in_offset=bass.IndirectOffsetOnAxis(
                            ap=src_sb[:pp, tt : tt + 1], axis=0
                        ),
                    )
                    nc.gpsimd.indirect_dma_start(
                        out=gd[:pp, i * C : (i + 1) * C],
                        out_offset=None,
                        in_=pd[:, :],
                        in_offset=bass.IndirectOffsetOnAxis(
                            ap=dst_sb[:pp, tt : tt + 1], axis=0
                        ),
                    )
                o = opool.tile([128, BLK_E], F32, tag="o")
                nc.vector.tensor_add(
                    out=gs[:, :blk_w], in0=gs[:, :blk_w], in1=gd[:, :blk_w]
                )
                nc.scalar.activation(
                    out=o[:, :blk_w],
                    in_=gs[:, :blk_w],
                    func=mybir.ActivationFunctionType.Sigmoid,
                )
                # full 128-row tiles in this block
                nfull = nt if (t + nt) * TILE_E <= E_C else nt - 1
                if nfull > 0:
                    r0 = t * TILE_E
                    nc.sync.dma_start(
                        out=out[r0 : r0 + nfull * 128, :].rearrange(
                            "(i p) c -> p i c", p=128
                        ),
                        in_=o[:, : nfull * C].rearrange("p (i c) -> p i c", c=C),
                    )
                if nfull < nt:  # trailing partial tile (64 edges)
                    i = nt - 1
                    tt = t + i
                    pp = E_C - tt * TILE_E
                    nc.sync.dma_start(
                        out=out[tt * TILE_E : tt * TILE_E + pp, :],
                        in_=o[:pp, i * C : i * C + C],
                    )
                t += nt


def _prep_inputs(h, src, dst, W, b):
    h = np.asarray(h, dtype=np.float32)
    src = np.asarray(src)
    dst = np.asarray(dst)
    W = np.asarray(W, dtype=np.float32)
    b = np.asarray(b, dtype=np.float32)

    ht = np.ascontiguousarray(h.T)                      # [128, 100000]
    wcat = np.ascontiguousarray(
        np.concatenate([W[:D], W[D:]], axis=1)          # [128, 256]
    )
    bcat = np.ascontiguousarray(
        np.tile(np.concatenate([np.zeros(C, np.float32), b])[None, :], (128, 1))
    )

    in_maps = []
    for c in range(N_CORES):
        s = src[c * E_C : (c + 1) * E_C].astype(np.int32)
        d = dst[c * E_C : (c + 1) * E_C].astype(np.int32)
        pad = N_TILES * TILE_E - E_C
        if pad:
            s = np.concatenate([s, np.zeros(pad, np.int32)])
            d = np.concatenate([d, np.zeros(pad, np.int32)])
        # [128, N_TILES]: element [p, t] = index of edge t*128 + p
        s2 = np.ascontiguousarray(s.reshape(N_TILES, 128).T)
        d2 = np.ascontiguousarray(d.reshape(N_TILES, 128).T)
        in_maps.append(
            {
                "ht": ht,
                "wcat": wcat,
                "bcat": bcat,
                "src_idx": s2,
                "dst_idx": d2,
            }
        )
    return in_maps


def kernel(h, src, dst, W, b):
    if "nc" not in _CACHE:
        t0 = time.time()
        _CACHE["nc"] = _build_program()
        if os.environ.get("KERNEL_VERBOSE"):
            print(f"[kernel] build+compile: {time.time() - t0:.1f}s")
    nc = _CACHE["nc"]
    in_maps = _prep_inputs(h, src, dst, W, b)
    res = run_bass_kernel_spmd(nc, in_maps, core_ids=list(range(N_CORES)))
    outs = [res.results[c]["out"] for c in range(N_CORES)]
    return np.concatenate(outs, axis=0)
